# revision 21
# baseline (speedup 1.0000x reference)
"""Trainium2 Bass kernel for Bottleneck_refine (masked grouped 1x1/3x3/1x1 conv + residual).

Strategy
--------
Spatial sharding: H=128 rows split into 8 slabs of 16 rows, one per NeuronCore.
Each core receives an 18-row slab (16 + 1-row halo each side, zero-padded at
image boundary) so the 3x3 conv needs no cross-core halo exchange.

Channel layout: channels are pre-permuted on host into "pair-packed" order:
for pair p in {0,1} (groups 2p, 2p+1) and j in 0..3, packed tile (4p+j) holds
64 channels of group 2p followed by 64 channels of group 2p+1. This makes every
matmul a clean K=128 / M=128 block-diagonal matmul with partition base 0:
  conv1: 4 accumulating K-chunk matmuls per N-chunk  -> pair-packed t1
  conv2: 9 shifted-offset accumulating matmuls (3x3 as 9 matmuls over a
         130-column zero-padded grid held in SBUF)
  conv3: block-diag matmul, then residual add (DVE) + Relu (ACT) + store,
         pipelined per 512-pixel chunk.
Masks: the block mask is constant across a group's channels at a fixed pixel,
so mask-mult commutes with grouped 1x1 convs and with relu:
  t1 = m . relu(conv1(x));  t2 = m . relu(conv2(t1));  out = relu(conv3(t2)+x)
The full-resolution mask planes are expanded on device from a tiny [4, 2340]
per-core table via one small matmul (selection matrix), then applied with
fused (max 0) * mask scalar_tensor_tensor ops on the vector engine.

Matmuls run in float32r (single-pass fp32 PE mode; fp32 proper is 4x slower).
"""

import numpy as np

G = 4
C_IN = 1024
H = 128
W = 128
NCORES = 8
SLAB = 16          # output rows per core
ROWS = SLAB + 2    # input slab rows incl halo
GCOLS = W + 2      # zero-padded column grid for conv2
NPIX = ROWS * GCOLS          # 2340
SLABPIX = ROWS * W           # 2304
OUTPIX = SLAB * W            # 2048

# packed channel permutation: packed index q = 128*(4p+j) + s
#   s <  64 -> original channel 512p + 64j + s          (group 2p)
#   s >= 64 -> original channel 512p + 256 + 64j + s-64 (group 2p+1)
def _perm():
    perm = np.empty(C_IN, dtype=np.int64)
    q = 0
    for p in range(2):
        for j in range(4):
            for s in range(128):
                if s < 64:
                    perm[q] = 512 * p + 64 * j + s
                else:
                    perm[q] = 512 * p + 256 + 64 * j + (s - 64)
                q += 1
    return perm

PERM = _perm()


def _pack_weights(w1, w2, w3):
    """Build block-diagonal lhsT weight tiles (see module docstring)."""
    W1 = np.asarray(w1)[:, :, 0, 0]          # [256 out, 256 in-per-group]
    W2 = np.asarray(w2)                      # [256 out, 64 in, 3, 3]
    W3 = np.asarray(w3)[:, :, 0, 0]          # [1024 out, 64 in]

    w1p = np.zeros((8, 128, 128), np.float32)
    w2p = np.zeros((18, 128, 128), np.float32)
    w3p = np.zeros((8, 128, 128), np.float32)
    for p in range(2):
        ga, gb = 2 * p, 2 * p + 1
        for j in range(4):
            # conv1 K-chunk j: rows = within-group inputs 64j..64j+64
            w1p[4 * p + j, 0:64, 0:64] = W1[ga * 64:(ga + 1) * 64, 64 * j:64 * (j + 1)].T
            w1p[4 * p + j, 64:128, 64:128] = W1[gb * 64:(gb + 1) * 64, 64 * j:64 * (j + 1)].T
            # conv3 M-chunk j: cols = within-group outputs 64j..64j+64
            w3p[4 * p + j, 0:64, 0:64] = W3[ga * 256 + 64 * j: ga * 256 + 64 * (j + 1), :].T
            w3p[4 * p + j, 64:128, 64:128] = W3[gb * 256 + 64 * j: gb * 256 + 64 * (j + 1), :].T
        for off in range(9):
            dy, dx = off // 3 - 1, off % 3 - 1
            w2p[9 * p + off, 0:64, 0:64] = W2[ga * 64:(ga + 1) * 64, :, dy + 1, dx + 1].T
            w2p[9 * p + off, 64:128, 64:128] = W2[gb * 64:(gb + 1) * 64, :, dy + 1, dx + 1].T

    bsel = np.zeros((2, 4, 128), np.float32)
    for p in range(2):
        bsel[p, 2 * p, 0:64] = 1.0
        bsel[p, 2 * p + 1, 64:128] = 1.0

    ident = np.eye(128, dtype=np.float32)
    return w1p, w2p, w3p, bsel, ident


def _core_mask_table(mask, h):
    """[4, NPIX] per-core full-res mask plane over the 18x130 slab grid."""
    m = np.asarray(mask)[0]                  # [4, 8, 8]
    tab = np.empty((4, ROWS, GCOLS), np.float32)
    rows = np.clip(np.arange(ROWS) + 16 * h - 1, 0, H - 1) // 16
    cols = np.clip(np.arange(GCOLS) - 1, 0, W - 1) // 16
    for g in range(4):
        tab[g] = m[g][np.ix_(rows, cols)]
    return tab.reshape(4, NPIX)


def _pack_cores(x, mask):
    """Per-core x slabs (pair-packed channels, halo rows) and mask tables."""
    xp = np.asarray(x)[0][PERM]                        # [1024, 128, 128]
    xp = np.pad(xp, ((0, 0), (1, 1), (0, 0)))          # [1024, 130, 128]
    slabs, mtabs = [], []
    for h in range(NCORES):
        slabs.append(np.ascontiguousarray(
            xp[:, 16 * h:16 * h + ROWS, :]).reshape(C_IN, SLABPIX))
        mtabs.append(_core_mask_table(mask, h))
    return slabs, mtabs


# ---------------------------------------------------------------------------
# numpy golden model of the device program (for validating packing/indexing)
# ---------------------------------------------------------------------------
def _golden_core(xs, mt, w1p, w2p, w3p, bsel, ident):
    """Emulate the device dataflow for one core. xs:[1024, SLABPIX] mt:[4, NPIX]."""
    out = np.zeros((C_IN, OUTPIX), np.float32)
    xtile = xs.reshape(8, 128, SLABPIX)
    for p in range(2):
        maskA = (bsel[p].T @ mt).reshape(128, ROWS, GCOLS)     # [128, 18, 130]
        # conv1 -> t1 on padded grid
        t1 = np.zeros((128, ROWS, GCOLS), np.float32)
        acc = np.zeros((128, SLABPIX), np.float32)
        for k in range(4):
            acc += w1p[4 * p + k].T @ xtile[4 * p + k]
        acc = np.maximum(acc, 0.0).reshape(128, ROWS, W)
        t1[:, :, 1:129] = acc * maskA[:, :, 1:129]
        # conv2 -> t2 (interior rows only)
        acc2 = np.zeros((128, SLAB, W), np.float32)
        for off in range(9):
            dy, dx = off // 3 - 1, off % 3 - 1
            sh = t1[:, 1 + dy:1 + dy + SLAB, 1 + dx:1 + dx + W]
            acc2 += np.einsum('km,kab->mab', w2p[9 * p + off], sh)
        t2 = (np.maximum(acc2, 0.0) * maskA[:, 1:17, 1:129]).reshape(128, OUTPIX)
        # conv3 + residual + relu
        xint = xtile[:, :, :].reshape(8, 128, ROWS, W)[:, :, 1:17, :].reshape(8, 128, OUTPIX)
        for j in range(4):
            acc3 = w3p[4 * p + j].T @ t2 + ident.T @ xint[4 * p + j]
            out[128 * (4 * p + j):128 * (4 * p + j + 1)] = np.maximum(acc3, 0.0)
    return out


def golden(x, mask, w1, w2, w3):
    """Full-pipeline numpy emulation -> full [1,1024,128,128] output."""
    w1p, w2p, w3p, bsel, ident = _pack_weights(w1, w2, w3)
    slabs, mtabs = _pack_cores(x, mask)
    out = np.zeros((1, C_IN, H, W), np.float32)
    for h in range(NCORES):
        o = _golden_core(slabs[h], mtabs[h], w1p, w2p, w3p, bsel, ident)
        out[0, PERM, 16 * h:16 * h + 16, :] = o.reshape(C_IN, SLAB, W)
    return out


# ---------------------------------------------------------------------------
# Bass program
# ---------------------------------------------------------------------------
_NC_CACHE = {}
PHASELOG = []   # (label, first_inst_idx, last_inst_idx) for trace attribution


def _build_nc():
    import concourse.bacc as bacc
    import concourse.mybir as mybir
    from concourse.tile import TileContext

    dt = mybir.dt
    f32 = dt.float32
    f32r = dt.float32r
    Relu = mybir.ActivationFunctionType.Relu
    Alu = mybir.AluOpType

    nc = bacc.Bacc(None, target_bir_lowering=False)
    # Everything that feeds a matmul is declared float32r end-to-end (same
    # 4-byte storage; the BIR verifier requires matmul operands to be
    # *produced* as f32r).
    x_d = nc.declare_dram_parameter('x', [C_IN, SLABPIX], f32r, isOutput=False)
    w1_d = nc.declare_dram_parameter('w1', [8, 128, 128], f32r, isOutput=False)
    w2_d = nc.declare_dram_parameter('w2', [18, 128, 128], f32r, isOutput=False)
    w3_d = nc.declare_dram_parameter('w3', [8, 128, 128], f32r, isOutput=False)
    bs_d = nc.declare_dram_parameter('bsel', [2, 4, 128], f32r, isOutput=False)
    mk_d = nc.declare_dram_parameter('msk', [4, NPIX], f32r, isOutput=False)
    out_d = nc.declare_dram_parameter('out', [C_IN, OUTPIX], f32, isOutput=True)

    r = lambda ap: ap

    with TileContext(nc) as tc:
        with (
            tc.tile_pool(name='const', bufs=1) as cpool,
            tc.tile_pool(name='xin', bufs=8) as xpool,
            tc.tile_pool(name='t1', bufs=2) as t1pool,
            tc.tile_pool(name='t2', bufs=2) as t2pool,
            tc.tile_pool(name='maskp', bufs=2) as mpool,
            tc.tile_pool(name='outp', bufs=12) as opool,
            tc.tile_pool(name='ps1', bufs=2, space='PSUM') as ps1pool,
            tc.tile_pool(name='ps2', bufs=2, space='PSUM') as ps2pool,
            tc.tile_pool(name='ps3', bufs=4, space='PSUM') as ps3pool,
        ):
            # ---- constants + x slabs, ordered so early-needed data arrives
            # first (bsel/msk for mask expansion, w1 + x tiles 0-3 for conv1
            # of pair 0; w2/w3 arrive while conv1 runs). x tiles are loaded
            # in two half-slabs so conv1 can start after ~half the load.
            bs_sb = cpool.tile([4, 2, 128], f32r, tag='bsel')
            nc.sync.dma_start(out=bs_sb[:], in_=bs_d.rearrange('t p c -> p t c'))
            mk_sb = cpool.tile([4, NPIX], f32r, tag='msk')
            nc.sync.dma_start(out=mk_sb[:], in_=mk_d[:, :])
            w1_sb = cpool.tile([128, 8, 128], f32r, tag='w1')
            nc.sync.dma_start(out=w1_sb[:], in_=w1_d.rearrange('t p c -> p t c'))

            # x loads split into conv1-chunk-aligned row pieces so conv1
            # starts as soon as the first rows land; w2/w3 interleaved at the
            # points the conv2/conv3 pipelines first need them.
            PIECES = [(0, 4), (4, 4), (8, 4), (12, 4), (16, 2)]
            xt = []
            for t in range(8):
                xtile = xpool.tile([128, SLABPIX], f32r, tag='x')
                xt.append(xtile)

            def load_x(tiles, pieces):
                for (r0, nr) in pieces:
                    for t in tiles:
                        nc.sync.dma_start(
                            out=xt[t][:, r0 * W:(r0 + nr) * W],
                            in_=x_d[128 * t:128 * (t + 1), r0 * W:(r0 + nr) * W])

            load_x(range(4), PIECES[:2])
            w2_sb = cpool.tile([128, 18, 128], f32r, tag='w2')
            nc.sync.dma_start(out=w2_sb[:], in_=w2_d.rearrange('t p c -> p t c'))
            load_x(range(4), PIECES[2:])
            w3_sb = cpool.tile([128, 8, 128], f32r, tag='w3')
            nc.sync.dma_start(out=w3_sb[:], in_=w3_d.rearrange('t p c -> p t c'))
            load_x(range(4, 8), PIECES)

            # ---- per-pair masks: maskA[p] = bsel[p].T @ msk ----
            maskA = []
            for p in range(2):
                ma = mpool.tile([128, NPIX], f32, tag=f'maskA{p}')
                for c0 in range(0, NPIX, 512):
                    n = min(512, NPIX - c0)
                    ps = ps1pool.tile([128, 512], f32, tag='ps1')
                    nc.tensor.matmul(ps[:, :n], r(bs_sb[:, p, :]), r(mk_sb[:, c0:c0 + n]),
                                     start=True, stop=True)
                    nc.vector.tensor_copy(ma[:, c0:c0 + n], ps[:, :n])
                maskA.append(ma)

            # Per-pair pipeline, software-interleaved at chunk granularity so
            # PE work (conv2) fills x-load wait windows and conv3 outputs
            # stream to HBM continuously:
            #   c1(0) c1(1) c2(0) c1(2) c2(1) c3(0) c1(3) c2(2) c3(1)
            #   c1(4) c2(3) c3(2) c3(3)
            row_chunks = [(0, 4), (4, 4), (8, 4), (12, 4), (16, 2)]
            for p in range(2):
                mav = maskA[p].rearrange('q (a b) -> q a b', b=GCOLS)
                t1 = t1pool.tile([128, NPIX], f32r, tag='t1')
                t1v = t1.rearrange('q (a b) -> q a b', b=GCOLS)
                # zero the W-pad columns (memset doesn't support f32r):
                # multiply a finite tile region by 0.0
                nc.vector.tensor_scalar_mul(t1v[:, :, 0:1], mav[:, :, 0:1], 0.0)
                nc.vector.tensor_scalar_mul(t1v[:, :, 129:130], mav[:, :, 129:130], 0.0)
                t2 = t2pool.tile([128, OUTPIX], f32r, tag='t2')
                t2v = t2.rearrange('q (a b) -> q a b', b=W)

                def conv1_chunk(ci, p=p, t1v=t1v, mav=mav):
                    r0, nr = row_chunks[ci]
                    n = nr * W
                    ps = ps1pool.tile([128, 512], f32, tag='ps1', name='c1ps')
                    for k in range(4):
                        nc.tensor.matmul(ps[:, :n], r(w1_sb[:, 4 * p + k, :]),
                                         r(xt[4 * p + k][:, r0 * W:(r0 + nr) * W]),
                                         start=(k == 0), stop=(k == 3))
                    nc.vector.scalar_tensor_tensor(
                        out=t1v[:, r0:r0 + nr, 1:129],
                        in0=ps[:, :n].rearrange('q (a b) -> q a b', b=W),
                        scalar=0.0, in1=mav[:, r0:r0 + nr, 1:129],
                        op0=Alu.max, op1=Alu.mult)

                def conv2_chunk(cc, p=p, t1v=t1v, t2v=t2v, mav=mav):
                    r0 = 1 + 4 * cc
                    ps = ps2pool.tile([128, 512], f32, tag='ps2', name='c2ps')
                    for off in range(9):
                        dy, dx = off // 3 - 1, off % 3 - 1
                        nc.tensor.matmul(
                            ps[:], r(w2_sb[:, 9 * p + off, :]),
                            r(t1v[:, r0 + dy:r0 + dy + 4, 1 + dx:1 + dx + W]),
                            start=(off == 0), stop=(off == 8))
                    nc.vector.scalar_tensor_tensor(
                        out=t2v[:, 4 * cc:4 * cc + 4, :],
                        in0=ps[:].rearrange('q (a b) -> q a b', b=W),
                        scalar=0.0, in1=mav[:, r0:r0 + 4, 1:129],
                        op0=Alu.max, op1=Alu.mult)

                def conv3_chunk(cc, p=p, t2=t2):
                    for j in range(4):
                        ps = ps3pool.tile([128, 512], f32, tag='ps3', name='c3ps')
                        nc.tensor.matmul(ps[:], r(w3_sb[:, 4 * p + j, :]),
                                         r(t2[:, 512 * cc:512 * (cc + 1)]),
                                         start=True, stop=True)
                        ot = opool.tile([128, 512], f32, tag='out', name='otile')
                        # residual add on DVE, relu on ACT, then store
                        nc.vector.tensor_add(
                            out=ot[:], in0=ps[:],
                            in1=xt[4 * p + j][:, W + 512 * cc:W + 512 * (cc + 1)].bitcast(f32))
                        nc.scalar.activation(ot[:], ot[:], Relu)
                        nc.sync.dma_start(
                            out=out_d[128 * (4 * p + j):128 * (4 * p + j + 1),
                                      512 * cc:512 * (cc + 1)],
                            in_=ot[:])

                def logged(label, fn, *a):
                    i0 = len(nc.inst_map)
                    fn(*a)
                    names = list(nc.inst_map)[i0:]
                    PHASELOG.append((f'p{p}.{label}', names))

                logged('c1.0', conv1_chunk, 0)
                logged('c1.1', conv1_chunk, 1)
                logged('c2.0', conv2_chunk, 0)
                logged('c1.2', conv1_chunk, 2)
                logged('c2.1', conv2_chunk, 1)
                logged('c3.0', conv3_chunk, 0)
                logged('c1.3', conv1_chunk, 3)
                logged('c2.2', conv2_chunk, 2)
                logged('c3.1', conv3_chunk, 1)
                logged('c1.4', conv1_chunk, 4)
                logged('c2.3', conv2_chunk, 3)
                logged('c3.2', conv3_chunk, 2)
                logged('c3.3', conv3_chunk, 3)
    nc.finalize()
    return nc


def _get_nc():
    if 'nc' not in _NC_CACHE:
        _NC_CACHE['nc'] = _build_nc()
    return _NC_CACHE['nc']


def kernel(x, mask, w1, w2, w3):
    from concourse.bass_utils import run_bass_kernel_spmd

    w1p, w2p, w3p, bsel, ident = _pack_weights(w1, w2, w3)
    slabs, mtabs = _pack_cores(x, mask)
    in_maps = []
    for h in range(NCORES):
        in_maps.append({
            'x': slabs[h], 'w1': w1p, 'w2': w2p, 'w3': w3p,
            'bsel': bsel, 'msk': mtabs[h],
        })
    nc = _get_nc()
    res = run_bass_kernel_spmd(nc, in_maps, list(range(NCORES))).results
    out = np.zeros((1, C_IN, H, W), np.float32)
    for h in range(NCORES):
        out[0, PERM, 16 * h:16 * h + 16, :] = res[h]['out'].reshape(C_IN, SLAB, W)
    return out
